# revision 18
# baseline (speedup 1.0000x reference)
"""Distributed causal multi-head attention for Trainium2 (8 NeuronCores).

Problem: B=2, T=2048, D=1024, 16 heads, head_dim=64, fp32 reference.
  q/k/v = x @ W{q,k,v}.T ; per-head causal softmax(q k^T/8) v ; out @ Wo.T

Sharding: tensor-parallel over heads — core c owns heads {2c, 2c+1}.
Per core (bf16 storage, fp32 PSUM accumulation):
  - Q^T, K^T = W_shard @ x^T   ([128 feat, 4096 tok])
  - V        = x @ Wv_shard.T  ([tok, feat] natural layout, with a fused
               ones-column at index 0 so PV also produces softmax denom)
  - scores as S^T [k, q] (K^T block stationary, Q^T moving, 2 heads
    row-packed at partition offsets 0/64 -> concurrent), exp on ACT
    (causal block skipping + sliced diagonal blocks), PV accumulates
    [sumexp; O^T].
  - normalize O^T by 1/sumexp (DVE recip at partition 0 + K=1 ones-matmul
    partition broadcast + DVE mult)
  - 4-round AllToAll (one per q-chunk, overlapped with attention):
    re-shard from feature-split to token-split; per-round o-proj with
    full Wo^T; each core outputs a [512, 1024] slice; host reassembles.
"""

import functools
import numpy as np
import ml_dtypes

import concourse.bass as bass
from concourse.bass import ds
import concourse.mybir as mybir
import concourse.tile as tile
from concourse import bacc
from concourse import bass_utils

F32 = mybir.dt.float32
F32R = mybir.dt.float32r
BF16 = mybir.dt.bfloat16

P = 128
NCORES = 8
B, T, DIM = 2, 2048, 1024
NH, HD = 16, 64
TOK = B * T               # 4096 flattened tokens
NKT = DIM // P            # 8 contraction tiles
NTC = TOK // 512          # 8 token chunks of 512
CPB = 4                   # q-chunks of 512 per batch
KPB = 16                  # 128-wide k-blocks per batch
SLICE = TOK // NCORES     # 512 output tokens per core


def build_kernel(debug=False):
    nc = bacc.Bacc("TRN2", num_devices=NCORES)

    xT = nc.declare_dram_parameter("xT", [NTC, P, NKT, 512], BF16, isOutput=False)
    wq = nc.declare_dram_parameter("wq", [P, NKT, P], BF16, isOutput=False)
    wk = nc.declare_dram_parameter("wk", [P, NKT, P], BF16, isOutput=False)
    wv = nc.declare_dram_parameter("wv", [P, NKT, P], BF16, isOutput=False)
    wo = nc.declare_dram_parameter("wo", [P, NKT, DIM], BF16, isOutput=False)
    tri = nc.declare_dram_parameter("tri", [P, P], F32, isOutput=False)
    out = nc.declare_dram_parameter("out", [SLICE, DIM], F32, isOutput=True)

    with tile.TileContext(nc) as tc:
        # ---- resident SBUF ----
        res = tc.alloc_tile_pool(name="res", bufs=1)
        QT = res.tile([P, NTC, 512], BF16, name="QT")      # [feat, tok]
        KT = res.tile([P, NTC, 512], BF16, name="KT")
        VA = res.tile([P, TOK // P, 2, HD + 1], BF16, name="VA")
        WQ = res.tile([P, NKT, P], BF16, name="WQ")
        WK = res.tile([P, NKT, P], BF16, name="WK")
        WV = res.tile([P, NKT, P], BF16, name="WV")
        WO = res.tile([P, NKT, DIM], BF16, name="WO")
        TRI = res.tile([P, P], F32, name="TRI")
        ONESF = res.tile([P, HD + 1], F32, name="ONESF")
        ONES = res.tile([P, HD + 1], F32R, name="ONES")

        nc.sync.dma_start(WQ[:], wq[:, :, :])
        nc.sync.dma_start(WK[:], wk[:, :, :])
        nc.sync.dma_start(WV[:], wv[:, :, :])
        nc.sync.dma_start(TRI[:], tri[:, :])
        nc.any.memset(ONESF[:], 1.0)
        nc.vector.tensor_copy(out=ONES[:], in_=ONESF[:])
        nc.any.memset(VA[:, :, :, 0:1], 1.0)

        # per-round A2A bounce buffers (bf16)
        a2a_in = []
        a2a_out = []
        frees = []
        for r in range(CPB):
            ai, f1 = tc.tile([P, 2 * 512], BF16, space="DRAM",
                             name=f"a2a_in{r}")
            ao, f2 = tc.tile([NCORES * P, 2 * 512], BF16, space="DRAM",
                             name=f"a2a_out{r}", addr_space="Shared")
            a2a_in.append(ai)
            a2a_out.append(ao)
            frees += [f1, f2]

        # ---- phase 1: QKV projections ----
        with (
            tc.tile_pool(name="xs", bufs=3) as xs,
            tc.tile_pool(name="qkv_ps", bufs=2, space="PSUM") as qkv_ps,
            tc.tile_pool(name="v_ps", bufs=2, space="PSUM") as v_ps,
        ):
            for tch in range(NTC):
                xt = xs.tile([P, NKT, 512], BF16, tag="xt")
                nc.sync.dma_start(xt[:], xT[tch, :, :, :])
                q_ps = qkv_ps.tile([P, 512], F32, tag="qk")
                for kt in range(NKT):
                    nc.tensor.matmul(q_ps[:], lhsT=WQ[:, kt, :], rhs=xt[:, kt, :],
                                     start=(kt == 0), stop=(kt == NKT - 1))
                nc.vector.tensor_copy(out=QT[:, tch, :], in_=q_ps[:])
                k_ps = qkv_ps.tile([P, 512], F32, tag="qk")
                for kt in range(NKT):
                    nc.tensor.matmul(k_ps[:], lhsT=WK[:, kt, :], rhs=xt[:, kt, :],
                                     start=(kt == 0), stop=(kt == NKT - 1))
                nc.vector.tensor_copy(out=KT[:, tch, :], in_=k_ps[:])
                # V natural layout: [tok 128, feat 128] per t-tile
                for tt4 in range(4):
                    tt = tch * 4 + tt4
                    vp = v_ps.tile([P, P], F32, tag="v")
                    for kt in range(NKT):
                        nc.tensor.matmul(
                            vp[:], lhsT=xt[:, kt, tt4 * P:(tt4 + 1) * P],
                            rhs=WV[:, kt, :],
                            start=(kt == 0), stop=(kt == NKT - 1))
                    nc.vector.tensor_copy(out=VA[:, tt, 0, 1:HD + 1],
                                          in_=vp[:, 0:HD])
                    nc.vector.tensor_copy(out=VA[:, tt, 1, 1:HD + 1],
                                          in_=vp[:, HD:P])
            # wo arrives last; only needed by phase-4 o-proj rounds
            nc.sync.dma_start(WO[:], wo[:, :, :])

        # ---- phase 2+3: attention with per-chunk A2A rounds + o-proj ----
        with (
            tc.tile_pool(name="sc_ps", bufs=2, space="PSUM") as sc_ps,
            tc.tile_pool(name="ot_ps", bufs=1, space="PSUM") as ot_ps,
            tc.tile_pool(name="rb_ps", bufs=1, space="PSUM") as rb_ps,
            tc.tile_pool(name="o_ps", bufs=1, space="PSUM") as o_ps_pool,
            tc.tile_pool(name="pt", bufs=4) as ptp,
            tc.tile_pool(name="nrm", bufs=3) as nrm,
            tc.tile_pool(name="att", bufs=2) as atp,
        ):
            pid = nc.sync.partition_id()

            def emit_oproj(c):
                att = atp.tile([P, NKT, P], BF16, tag="att")
                for kt in range(NKT):
                    nc.sync.dma_start(att[:, kt, :],
                                      a2a_out[c][kt * P:(kt + 1) * P,
                                                 ds(pid * P, P)])
                for oh in range(2):
                    op = o_ps_pool.tile([P, 512], F32, tag="o")
                    for kt in range(NKT):
                        nc.tensor.matmul(
                            op[:], lhsT=att[:, kt, :],
                            rhs=WO[:, kt, oh * 512:(oh + 1) * 512],
                            start=(kt == 0), stop=(kt == NKT - 1))
                    osb = atp.tile([P, 512], F32, tag="osb")
                    nc.vector.tensor_copy(out=osb[:], in_=op[:])
                    nc.sync.dma_start(
                        out[c * P:(c + 1) * P, oh * 512:(oh + 1) * 512],
                        osb[:])

            for c in range(CPB):
                for b in range(B):
                    qch = b * CPB + c
                    oA = ot_ps.tile([HD + 1, 512], F32, tag="oA")
                    oB = ot_ps.tile([HD + 1, 512], F32, tag="oB")
                    nkb = 4 * (c + 1)

                    def emit_scores(kb):
                        kch = b * CPB + kb // 4
                        kcol = (kb % 4) * P
                        off = max(0, kb - 4 * c) * P  # first valid q column
                        s_ps = sc_ps.tile([P, 2, 512], F32, tag="s")
                        nc.tensor.matmul(
                            s_ps[:, 0, off:512],
                            lhsT=KT[0:HD, kch, kcol:kcol + P],
                            rhs=QT[0:HD, qch, off:512], start=True, stop=True)
                        nc.tensor.matmul(
                            s_ps[:, 1, off:512],
                            lhsT=KT[HD:P, kch, kcol:kcol + P],
                            rhs=QT[HD:P, qch, off:512], start=True, stop=True,
                            tile_position=(HD, 0))
                        return s_ps

                    def emit_exp(kb, s_ps):
                        d = kb - 4 * c
                        off = max(0, d) * P
                        if d >= 0:
                            nc.vector.tensor_add(
                                out=s_ps[:, :, off:off + P],
                                in0=s_ps[:, :, off:off + P],
                                in1=TRI[:, None, :].to_broadcast([P, 2, P]))
                        pt = ptp.tile([P, 2, 512], BF16, tag="p")
                        nc.scalar.activation(
                            pt[:, :, off:512], s_ps[:, :, off:512],
                            mybir.ActivationFunctionType.Exp, scale=0.125)
                        return pt

                    s_tiles = {0: emit_scores(0)}
                    if nkb > 1:
                        s_tiles[1] = emit_scores(1)
                    pt_tiles = {0: emit_exp(0, s_tiles.pop(0))}
                    for kb in range(nkb):
                        if kb + 1 < nkb:
                            pt_tiles[kb + 1] = emit_exp(kb + 1,
                                                        s_tiles.pop(kb + 1))
                        if kb + 2 < nkb:
                            s_tiles[kb + 2] = emit_scores(kb + 2)
                        off = max(0, kb - 4 * c) * P
                        pt = pt_tiles.pop(kb)
                        ktile = b * KPB + kb
                        nc.tensor.matmul(oA[:, off:512],
                                         lhsT=VA[:, ktile, 0, :],
                                         rhs=pt[:, 0, off:512],
                                         start=(kb == 0), stop=(kb == nkb - 1))
                        nc.tensor.matmul(oB[:, off:512],
                                         lhsT=VA[:, ktile, 1, :],
                                         rhs=pt[:, 1, off:512],
                                         start=(kb == 0), stop=(kb == nkb - 1))
                    # normalize + scatter into round-c bounce buffer
                    for h, o_ps in ((0, oA), (1, oB)):
                        rr = nrm.tile([1, 512], F32, tag="rr")
                        nc.vector.tensor_copy(out=rr[:], in_=o_ps[0:1, :])
                        nc.vector.reciprocal_approx_fast(out=rr[:], in_=rr[:])
                        rrr = nrm.tile([1, 512], F32R, tag="rrr")
                        nc.vector.tensor_copy(out=rrr[:], in_=rr[:])
                        rb = rb_ps.tile([HD + 1, 512], F32, tag="rb")
                        nc.tensor.matmul(rb[:], lhsT=ONES[0:1, :],
                                         rhs=rrr[:], start=True, stop=True)
                        rbs = nrm.tile([HD + 1, 512], F32, tag="rbs")
                        nc.vector.tensor_copy(out=rbs[:], in_=rb[:])
                        onrm = nrm.tile([HD + 1, 512], BF16, tag="on")
                        nc.vector.tensor_mul(out=onrm[:], in0=o_ps[:],
                                             in1=rbs[:])
                        nc.sync.dma_start(
                            a2a_in[c][h * HD:(h + 1) * HD,
                                      512 * b:512 * b + 512],
                            onrm[1:HD + 1, :])
                # round-c collective (o-proj emitted later, pipelined)
                if c > 0:
                    emit_oproj(c - 1)
                nc.gpsimd.collective_compute(
                    "AllGather", mybir.AluOpType.bypass,
                    replica_groups=[list(range(NCORES))],
                    ins=[a2a_in[c][:, :].opt()],
                    outs=[a2a_out[c][:, :].opt()],
                )
            emit_oproj(CPB - 1)

        for f in frees:
            f()
        res.release()
    nc.finalize()
    return nc


@functools.cache
def _get_nc():
    return build_kernel()


def _bf(a):
    return np.asarray(a, np.float32).astype(ml_dtypes.bfloat16)


def _prep_w(w_shard):
    # [128 out-feat, 1024 in] -> lhsT tiles [p, kt, m]: w[p,kt,m]=W[m, kt*128+p]
    return np.ascontiguousarray(
        _bf(w_shard).T.reshape(NKT, P, w_shard.shape[0]).transpose(1, 0, 2))


_last_in_maps = None


def kernel(x, mask, Wq, Wk, Wv, Wo):
    x = np.asarray(x, np.float32)

    xt = _bf(x).reshape(TOK, DIM).T            # [D, TOK] bf16
    xt = np.ascontiguousarray(xt).reshape(NKT, P, NTC, 512)
    xT = np.ascontiguousarray(xt.transpose(2, 1, 0, 3))  # [tch, p, kt, 512]
    wo_t = np.ascontiguousarray(
        _bf(Wo).T.reshape(NKT, P, DIM).transpose(1, 0, 2))
    tri = np.where(np.triu(np.ones((P, P), np.bool_)), 0.0,
                   -1e30).astype(np.float32)

    in_maps = []
    for c in range(NCORES):
        sl = slice(c * P, (c + 1) * P)
        in_maps.append(dict(
            xT=xT,
            wq=_prep_w(np.asarray(Wq, np.float32)[sl]),
            wk=_prep_w(np.asarray(Wk, np.float32)[sl]),
            wv=_prep_w(np.asarray(Wv, np.float32)[sl]),
            wo=wo_t,
            tri=tri,
        ))

    nc = _get_nc()
    global _last_in_maps
    _last_in_maps = in_maps
    res = bass_utils.run_bass_kernel_spmd(nc, in_maps,
                                          core_ids=list(range(NCORES)))
    full = np.empty((TOK, DIM), np.float32)
    for j in range(NCORES):
        o = res.results[j]["out"]          # [512, 1024], rows = 4 rounds x 128
        bb = j // 4
        for c in range(CPB):
            t0 = 512 * c + P * (j % 4)
            full[bb * T + t0: bb * T + t0 + P] = o[c * P:(c + 1) * P]
    return full.reshape(B, T, DIM)


if __name__ == "__main__":
    rng = np.random.default_rng(0)
    x = rng.standard_normal((B, T, DIM)).astype(np.float32)
    neg = np.finfo(np.float32).min
    mask = np.triu(np.full((T, T), neg, np.float32), k=1)[None, None]
    Ws = [(rng.standard_normal((DIM, DIM)) * 0.02).astype(np.float32)
          for _ in range(4)]
    out = kernel(x, mask, *Ws)
    print("out", out.shape, out.dtype, np.abs(out).max())


# revision 19
# speedup vs baseline: 1.1973x; 1.1973x over previous
"""Distributed causal multi-head attention for Trainium2 (8 NeuronCores).

Problem: B=2, T=2048, D=1024, 16 heads, head_dim=64, fp32 reference.
  q/k/v = x @ W{q,k,v}.T ; per-head causal softmax(q k^T/8) v ; out @ Wo.T

Sharding: tensor-parallel over heads — core c owns heads {2c, 2c+1}.
Per core (bf16 storage, fp32 PSUM accumulation):
  - Q^T, K^T = W_shard @ x^T   ([128 feat, 4096 tok])
  - V        = x @ Wv_shard.T  ([tok, feat] natural layout, with a fused
               ones-column at index 0 so PV also produces softmax denom)
  - scores as S^T [k, q] (K^T block stationary, Q^T moving, 2 heads
    row-packed at partition offsets 0/64 -> concurrent), exp on ACT
    (causal block skipping + sliced diagonal blocks), PV accumulates
    [sumexp; O^T].
  - normalize O^T by 1/sumexp (DVE recip at partition 0 + K=1 ones-matmul
    partition broadcast + DVE mult)
  - 4-round AllToAll (one per q-chunk, overlapped with attention):
    re-shard from feature-split to token-split; per-round o-proj with
    full Wo^T; each core outputs a [512, 1024] slice; host reassembles.
"""

import functools
import numpy as np
import ml_dtypes

import concourse.bass as bass
from concourse.bass import ds
import concourse.mybir as mybir
import concourse.tile as tile
from concourse import bacc
from concourse import bass_utils

F32 = mybir.dt.float32
F32R = mybir.dt.float32r
BF16 = mybir.dt.bfloat16

P = 128
NCORES = 8
B, T, DIM = 2, 2048, 1024
NH, HD = 16, 64
TOK = B * T               # 4096 flattened tokens
NKT = DIM // P            # 8 contraction tiles
NTC = TOK // 512          # 8 token chunks of 512
CPB = 4                   # q-chunks of 512 per batch
KPB = 16                  # 128-wide k-blocks per batch
SLICE = TOK // NCORES     # 512 output tokens per core


def build_kernel(debug=False):
    nc = bacc.Bacc("TRN2", num_devices=NCORES)

    xT = nc.declare_dram_parameter("xT", [NTC, P, NKT, 512], BF16, isOutput=False)
    wq = nc.declare_dram_parameter("wq", [P, NKT, P], BF16, isOutput=False)
    wk = nc.declare_dram_parameter("wk", [P, NKT, P], BF16, isOutput=False)
    wv = nc.declare_dram_parameter("wv", [P, NKT, P], BF16, isOutput=False)
    wo = nc.declare_dram_parameter("wo", [P, NKT, DIM], BF16, isOutput=False)
    tri = nc.declare_dram_parameter("tri", [P, P], F32, isOutput=False)
    out = nc.declare_dram_parameter("out", [SLICE, DIM], F32, isOutput=True)

    with tile.TileContext(nc) as tc:
        # ---- resident SBUF ----
        res = tc.alloc_tile_pool(name="res", bufs=1)
        QT = res.tile([P, NTC, 512], BF16, name="QT")      # [feat, tok]
        KT = res.tile([P, NTC, 512], BF16, name="KT")
        VA = res.tile([P, TOK // P, 2, HD + 1], BF16, name="VA")
        WQ = res.tile([P, NKT, P], BF16, name="WQ")
        WK = res.tile([P, NKT, P], BF16, name="WK")
        WV = res.tile([P, NKT, P], BF16, name="WV")
        WO = res.tile([P, NKT, DIM], BF16, name="WO")
        TRI = res.tile([P, P], F32, name="TRI")
        ONESF = res.tile([P, HD + 1], F32, name="ONESF")
        ONES = res.tile([P, HD + 1], F32R, name="ONES")

        nc.sync.dma_start(WQ[:], wq[:, :, :])
        nc.sync.dma_start(WK[:], wk[:, :, :])
        nc.sync.dma_start(WV[:], wv[:, :, :])
        nc.sync.dma_start(TRI[:], tri[:, :])
        nc.any.memset(ONESF[:], 1.0)
        nc.vector.tensor_copy(out=ONES[:], in_=ONESF[:])
        nc.any.memset(VA[:, :, :, 0:1], 1.0)

        # per-round A2A bounce buffers (bf16)
        a2a_in = []
        a2a_out = []
        frees = []
        for r in range(CPB):
            ai, f1 = tc.tile([NCORES * P, P], BF16, space="DRAM",
                             name=f"a2a_in{r}")
            ao, f2 = tc.tile([NCORES * P, P], BF16, space="DRAM",
                             name=f"a2a_out{r}", addr_space="Shared")
            a2a_in.append(ai)
            a2a_out.append(ao)
            frees += [f1, f2]

        # ---- phase 1: QKV projections ----
        with (
            tc.tile_pool(name="xs", bufs=3) as xs,
            tc.tile_pool(name="qkv_ps", bufs=2, space="PSUM") as qkv_ps,
            tc.tile_pool(name="v_ps", bufs=2, space="PSUM") as v_ps,
        ):
            for tch in range(NTC):
                xt = xs.tile([P, NKT, 512], BF16, tag="xt")
                nc.sync.dma_start(xt[:], xT[tch, :, :, :])
                q_ps = qkv_ps.tile([P, 512], F32, tag="qk")
                for kt in range(NKT):
                    nc.tensor.matmul(q_ps[:], lhsT=WQ[:, kt, :], rhs=xt[:, kt, :],
                                     start=(kt == 0), stop=(kt == NKT - 1))
                nc.vector.tensor_copy(out=QT[:, tch, :], in_=q_ps[:])
                k_ps = qkv_ps.tile([P, 512], F32, tag="qk")
                for kt in range(NKT):
                    nc.tensor.matmul(k_ps[:], lhsT=WK[:, kt, :], rhs=xt[:, kt, :],
                                     start=(kt == 0), stop=(kt == NKT - 1))
                nc.vector.tensor_copy(out=KT[:, tch, :], in_=k_ps[:])
                # V natural layout: [tok 128, feat 128] per t-tile
                for tt4 in range(4):
                    tt = tch * 4 + tt4
                    vp = v_ps.tile([P, P], F32, tag="v")
                    for kt in range(NKT):
                        nc.tensor.matmul(
                            vp[:], lhsT=xt[:, kt, tt4 * P:(tt4 + 1) * P],
                            rhs=WV[:, kt, :],
                            start=(kt == 0), stop=(kt == NKT - 1))
                    nc.vector.tensor_copy(out=VA[:, tt, 0, 1:HD + 1],
                                          in_=vp[:, 0:HD])
                    nc.vector.tensor_copy(out=VA[:, tt, 1, 1:HD + 1],
                                          in_=vp[:, HD:P])
            # wo arrives last; only needed by phase-4 o-proj rounds
            nc.sync.dma_start(WO[:], wo[:, :, :])

        # ---- phase 2+3: attention with per-chunk A2A rounds + o-proj ----
        with (
            tc.tile_pool(name="sc_ps", bufs=2, space="PSUM") as sc_ps,
            tc.tile_pool(name="ot_ps", bufs=1, space="PSUM") as ot_ps,
            tc.tile_pool(name="rb_ps", bufs=1, space="PSUM") as rb_ps,
            tc.tile_pool(name="o_ps", bufs=1, space="PSUM") as o_ps_pool,
            tc.tile_pool(name="pt", bufs=4) as ptp,
            tc.tile_pool(name="nrm", bufs=3) as nrm,
            tc.tile_pool(name="att", bufs=2) as atp,
        ):
            pid = nc.sync.partition_id()

            def emit_oproj(c):
                att = atp.tile([P, NKT, P], BF16, tag="att")
                for kt in range(NKT):
                    nc.sync.dma_start(att[:, kt, :],
                                      a2a_out[c][kt * P:(kt + 1) * P, :])
                for oh in range(2):
                    op = o_ps_pool.tile([P, 512], F32, tag="o")
                    for kt in range(NKT):
                        nc.tensor.matmul(
                            op[:], lhsT=att[:, kt, :],
                            rhs=WO[:, kt, oh * 512:(oh + 1) * 512],
                            start=(kt == 0), stop=(kt == NKT - 1))
                    osb = atp.tile([P, 512], F32, tag="osb")
                    nc.vector.tensor_copy(out=osb[:], in_=op[:])
                    nc.sync.dma_start(
                        out[c * P:(c + 1) * P, oh * 512:(oh + 1) * 512],
                        osb[:])

            for c in range(CPB):
                for b in range(B):
                    qch = b * CPB + c
                    oA = ot_ps.tile([HD + 1, 512], F32, tag="oA")
                    oB = ot_ps.tile([HD + 1, 512], F32, tag="oB")
                    nkb = 4 * (c + 1)

                    def emit_scores(kb):
                        kch = b * CPB + kb // 4
                        kcol = (kb % 4) * P
                        off = max(0, kb - 4 * c) * P  # first valid q column
                        s_ps = sc_ps.tile([P, 2, 512], F32, tag="s")
                        nc.tensor.matmul(
                            s_ps[:, 0, off:512],
                            lhsT=KT[0:HD, kch, kcol:kcol + P],
                            rhs=QT[0:HD, qch, off:512], start=True, stop=True)
                        nc.tensor.matmul(
                            s_ps[:, 1, off:512],
                            lhsT=KT[HD:P, kch, kcol:kcol + P],
                            rhs=QT[HD:P, qch, off:512], start=True, stop=True,
                            tile_position=(HD, 0))
                        return s_ps

                    def emit_exp(kb, s_ps):
                        d = kb - 4 * c
                        off = max(0, d) * P
                        if d >= 0:
                            nc.vector.tensor_add(
                                out=s_ps[:, :, off:off + P],
                                in0=s_ps[:, :, off:off + P],
                                in1=TRI[:, None, :].to_broadcast([P, 2, P]))
                        pt = ptp.tile([P, 2, 512], BF16, tag="p")
                        nc.scalar.activation(
                            pt[:, :, off:512], s_ps[:, :, off:512],
                            mybir.ActivationFunctionType.Exp, scale=0.125)
                        return pt

                    s_tiles = {0: emit_scores(0)}
                    if nkb > 1:
                        s_tiles[1] = emit_scores(1)
                    pt_tiles = {0: emit_exp(0, s_tiles.pop(0))}
                    for kb in range(nkb):
                        if kb + 1 < nkb:
                            pt_tiles[kb + 1] = emit_exp(kb + 1,
                                                        s_tiles.pop(kb + 1))
                        if kb + 2 < nkb:
                            s_tiles[kb + 2] = emit_scores(kb + 2)
                        off = max(0, kb - 4 * c) * P
                        pt = pt_tiles.pop(kb)
                        ktile = b * KPB + kb
                        nc.tensor.matmul(oA[:, off:512],
                                         lhsT=VA[:, ktile, 0, :],
                                         rhs=pt[:, 0, off:512],
                                         start=(kb == 0), stop=(kb == nkb - 1))
                        nc.tensor.matmul(oB[:, off:512],
                                         lhsT=VA[:, ktile, 1, :],
                                         rhs=pt[:, 1, off:512],
                                         start=(kb == 0), stop=(kb == nkb - 1))
                    # normalize + scatter into round-c bounce buffer
                    for h, o_ps in ((0, oA), (1, oB)):
                        rr = nrm.tile([1, 512], F32, tag="rr")
                        nc.vector.tensor_copy(out=rr[:], in_=o_ps[0:1, :])
                        nc.vector.reciprocal_approx_fast(out=rr[:], in_=rr[:])
                        rrr = nrm.tile([1, 512], F32R, tag="rrr")
                        nc.vector.tensor_copy(out=rrr[:], in_=rr[:])
                        rb = rb_ps.tile([HD + 1, 512], F32, tag="rb")
                        nc.tensor.matmul(rb[:], lhsT=ONES[0:1, :],
                                         rhs=rrr[:], start=True, stop=True)
                        rbs = nrm.tile([HD + 1, 512], F32, tag="rbs")
                        nc.vector.tensor_copy(out=rbs[:], in_=rb[:])
                        onrm = nrm.tile([HD + 1, 512], BF16, tag="on")
                        nc.vector.tensor_mul(out=onrm[:], in0=o_ps[:],
                                             in1=rbs[:])
                        for i in range(4):
                            nc.sync.dma_start(
                                a2a_in[c][(4 * b + i) * P + h * HD:
                                          (4 * b + i) * P + (h + 1) * HD, :],
                                onrm[1:HD + 1, i * P:(i + 1) * P])
                # round-c collective (o-proj deferred two rounds)
                if c >= 2:
                    emit_oproj(c - 2)
                nc.gpsimd.collective_compute(
                    "AllToAll", mybir.AluOpType.bypass,
                    replica_groups=[list(range(NCORES))],
                    ins=[a2a_in[c][:, :].opt()],
                    outs=[a2a_out[c][:, :].opt()],
                )
            emit_oproj(CPB - 2)
            emit_oproj(CPB - 1)

        for f in frees:
            f()
        res.release()
    nc.finalize()
    return nc


@functools.cache
def _get_nc():
    return build_kernel()


def _bf(a):
    return np.asarray(a, np.float32).astype(ml_dtypes.bfloat16)


def _prep_w(w_shard):
    # [128 out-feat, 1024 in] -> lhsT tiles [p, kt, m]: w[p,kt,m]=W[m, kt*128+p]
    return np.ascontiguousarray(
        _bf(w_shard).T.reshape(NKT, P, w_shard.shape[0]).transpose(1, 0, 2))


_last_in_maps = None


def kernel(x, mask, Wq, Wk, Wv, Wo):
    x = np.asarray(x, np.float32)

    xt = _bf(x).reshape(TOK, DIM).T            # [D, TOK] bf16
    xt = np.ascontiguousarray(xt).reshape(NKT, P, NTC, 512)
    xT = np.ascontiguousarray(xt.transpose(2, 1, 0, 3))  # [tch, p, kt, 512]
    wo_t = np.ascontiguousarray(
        _bf(Wo).T.reshape(NKT, P, DIM).transpose(1, 0, 2))
    tri = np.where(np.triu(np.ones((P, P), np.bool_)), 0.0,
                   -1e30).astype(np.float32)

    in_maps = []
    for c in range(NCORES):
        sl = slice(c * P, (c + 1) * P)
        in_maps.append(dict(
            xT=xT,
            wq=_prep_w(np.asarray(Wq, np.float32)[sl]),
            wk=_prep_w(np.asarray(Wk, np.float32)[sl]),
            wv=_prep_w(np.asarray(Wv, np.float32)[sl]),
            wo=wo_t,
            tri=tri,
        ))

    nc = _get_nc()
    global _last_in_maps
    _last_in_maps = in_maps
    res = bass_utils.run_bass_kernel_spmd(nc, in_maps,
                                          core_ids=list(range(NCORES)))
    full = np.empty((TOK, DIM), np.float32)
    for j in range(NCORES):
        o = res.results[j]["out"]          # [512, 1024], rows = 4 rounds x 128
        bb = j // 4
        for c in range(CPB):
            t0 = 512 * c + P * (j % 4)
            full[bb * T + t0: bb * T + t0 + P] = o[c * P:(c + 1) * P]
    return full.reshape(B, T, DIM)


if __name__ == "__main__":
    rng = np.random.default_rng(0)
    x = rng.standard_normal((B, T, DIM)).astype(np.float32)
    neg = np.finfo(np.float32).min
    mask = np.triu(np.full((T, T), neg, np.float32), k=1)[None, None]
    Ws = [(rng.standard_normal((DIM, DIM)) * 0.02).astype(np.float32)
          for _ in range(4)]
    out = kernel(x, mask, *Ws)
    print("out", out.shape, out.dtype, np.abs(out).max())
